# revision 2
# baseline (speedup 1.0000x reference)
"""HTSubTree forward as a distributed Bass kernel on 8 TRN2 NeuronCores.

out[b,u,v,r] = sum_{i,j,p} x[b,(i,j)] * WL[i,u,p] * WR2[j,v,p,r]
  (WL = f0*f1*c_left, WR2 = f2*f3*c_right*c_root, host-precontracted.)
Pure batch data-parallelism: 64 of 512 batch elements per core.

Design notes (from the v1 NTFF trace, 90.6us):
  - v1 ran every matmul at the 1.2GHz mid p-state: the in-order PE
    stalled each pair waiting for the DVE/ACT relayout, which reset the
    DVFS busy-streak (full 2.4GHz needs ~3us uninterrupted).
  - GPSIMD cannot touch PSUM (verifier), so PSUM eviction has only
    DVE+ACT. The v1/v2-style par-crossing relayout (4x [64,256] copies)
    plus the out eviction exceeds the PE pair budget -> would stall PE.
  - v3 instead has stage1 write y2 DIRECTLY in the (par,j)-partition
    layout via 4 matmuls with strided PSUM out APs (b2 x par, K=64,
    M=64, N=256). Costs +512 PE rows/pair but drops eviction to two
    contiguous 128-partition copies per pair (y2 cast->bf16, out
    cast->bf16), one on DVE + one on ACT, ~650ns each << PE pair time.
    PE is then the sole, continuously-busy bottleneck at full p-state.
  - all operands bf16 (tolerance 2e-2, this lands ~2e-3), warmup
    matmuls bridge the weight-DMA window, out DMA'd once per 4 pairs.

Per pair (local b0 = 2k, b1 = 2k+1):
  stage1 (x4, b2 x par): y2p[(par,j), strided (c,b2,u)] =
      x_b2[i,j].T @ wlf[i, par*256:(par+1)*256]          K=64 M=64 N=256
  y2 evict (DVE/ACT alternating): y2[(par,j), (c,b2,u)] bf16 <- y2p
  stage2 (x4 accum): po[(b2,u), (v,r)] += y2[:, c].T @ wr2c[c]  K=128 N=512
  out evict (other engine): ot[:, pair] bf16 <- po; DMA per 4-pair group.
"""

import sys

sys.path.insert(0, "/opt/trn_rl_repo")

import numpy as np
import ml_dtypes

import concourse.bass as bass
import concourse.tile as tile
from concourse import bacc, mybir
from concourse.bass_utils import run_bass_kernel_spmd

NCORES = 8
B = 512
BLOC = B // NCORES  # 64 batch elements per core
F32 = mybir.dt.float32
BF16 = mybir.dt.bfloat16
BF16_NP = ml_dtypes.bfloat16

NPAIR = 32
NWARM = 8

_COMPILED = None


def _build():
    nc = bacc.Bacc("TRN2", target_bir_lowering=False, debug=False)
    x_ap = nc.dram_tensor("x", [64, BLOC * 64], BF16, kind="ExternalInput").ap()
    wlf_ap = nc.dram_tensor("wlf", [64, 512], BF16, kind="ExternalInput").ap()
    wr2_ap = nc.dram_tensor("wr2all", [128, 2048], BF16, kind="ExternalInput").ap()
    out_ap = nc.dram_tensor("out", [BLOC * 64, 512], BF16, kind="ExternalOutput").ap()

    with tile.TileContext(nc) as tc:
        with (
            tc.tile_pool(name="weights", bufs=1) as wpool,
            tc.tile_pool(name="xin", bufs=3) as xpool,
            tc.tile_pool(name="y2", bufs=3) as ypool,
            tc.tile_pool(name="ostage", bufs=3) as opool,
            tc.tile_pool(name="py", bufs=2, space="PSUM") as pypool,
            tc.tile_pool(name="po", bufs=3, space="PSUM") as popool,
            tc.tile_pool(name="scr", bufs=1, space="PSUM") as spool,
        ):
            # -- warmup operands: memset, no DMA dependency --
            dummy = wpool.tile([64, 512], BF16, tag="dummy")
            nc.gpsimd.memset(dummy[:], 0.125)

            # -- input DMAs, ordered by when the PE needs them --
            xts = {}

            def load_group(g):
                t = xpool.tile([64, 512], BF16, tag="xg", name="xg")
                nc.sync.dma_start(t[:], x_ap[:, g * 512:(g + 1) * 512])
                xts[g] = t

            load_group(0)
            wlf = wpool.tile([64, 512], BF16, tag="wlf")
            nc.sync.dma_start(wlf[:], wlf_ap[:])
            wr2all = wpool.tile([128, 2048], BF16, tag="wr2all")
            nc.sync.dma_start(wr2all[:, 0:1024], wr2_ap[:, 0:1024])
            nc.sync.dma_start(wr2all[:, 1024:2048], wr2_ap[:, 1024:2048])
            wr2 = [wr2all[:, c * 512:(c + 1) * 512] for c in range(4)]

            # -- warmup: PE busy-streak from ~6.5us (DVFS ramp) --
            scr = spool.tile([128, 512], F32, tag="scr", space="PSUM")
            for _w in range(NWARM):
                nc.tensor.matmul(scr[:], dummy[:, 0:128], dummy[:], start=True,
                                 stop=True)

            y2p = {}
            y2t = {}
            pot = {}
            ott = {}

            def stage1(p):
                g = p // 4
                if p % 4 == 0 and g + 1 < 8:
                    load_group(g + 1)
                t = pypool.tile([128, 512], F32, tag="y2p", space="PSUM", name="y2p")
                for b2 in range(2):
                    xb = xts[g][:, (p % 4) * 128 + b2 * 64:(p % 4) * 128 + b2 * 64 + 64]
                    for par in range(2):
                        dst = t[par * 64:(par + 1) * 64].rearrange(
                            "j (c b2 u) -> b2 j c u", c=4, b2=2, u=64)[b2]
                        nc.tensor.matmul(dst, xb,
                                         wlf[:, par * 256:(par + 1) * 256],
                                         start=True, stop=True)
                y2p[p] = t

            def y2_evict(p):
                y2 = ypool.tile([128, 512], BF16, tag="y2", name="y2")
                eng = nc.vector.tensor_copy if p % 2 == 0 else nc.scalar.copy
                eng(y2[:], y2p[p][:])
                y2t[p] = y2

            def stage2(p):
                po = popool.tile([128, 512], F32, tag="po", space="PSUM", name="po")
                for c in range(4):
                    nc.tensor.matmul(po[:], y2t[p][:, c * 128:(c + 1) * 128],
                                     wr2[c][:], start=(c == 0), stop=(c == 3))
                pot[p] = po

            def out_evict(p):
                h = p // 2
                eng = nc.scalar.copy if p % 2 == 0 else nc.vector.tensor_copy
                if p >= NPAIR - 2:
                    ott[p] = opool.tile([128, 512], BF16, tag="otl", name="otl")
                    eng(ott[p][:], pot[p][:])
                    return
                if p % 2 == 0:
                    ott[h] = opool.tile([128, 1024], BF16, tag="ot", name="ot")
                q = p % 2
                eng(ott[h][:, q * 512:(q + 1) * 512], pot[p][:])

            def out_dma(h):
                dst = out_ap[h * 256:(h + 1) * 256].rearrange(
                    "(p q) n -> q p n", p=2, q=128)
                src = ott[h].rearrange("q (p n) -> q p n", p=2)
                nc.sync.dma_start(dst, src)

            def out_dma_last(p):
                nc.sync.dma_start(out_ap[p * 128:(p + 1) * 128, :], ott[p][:])

            # -- software-pipelined pair loop: PE program order is
            #    s1(0), s1(1), s2(0), s1(2), s2(1), ... so the y2 eviction
            #    of pair p runs under s1(p+1)+s2(p-1): PE never stalls.
            stage1(0)
            for p in range(NPAIR):
                if p + 1 < NPAIR:
                    stage1(p + 1)
                y2_evict(p)
                stage2(p)
                out_evict(p)
                if p >= NPAIR - 2:
                    out_dma_last(p)
                elif p % 2 == 1:
                    out_dma(p // 2)

    nc.compile()
    return nc


def _host_prep(x, factors, cores):
    """Pre-contract the tiny parameters and lay out per-core shards."""
    f0, f1, f2, f3 = factors[0], factors[1], factors[2], factors[3]
    c_root, c_left, c_right = cores[0], cores[1], cores[2]
    # WL[(i0,i1),(o0,o1),p=r02]
    wl = np.einsum("ioa,jpb,abr->ijopr", f0, f1, c_left, optimize=True)
    wl = wl.reshape(64, 64, 8)  # [i, u, p]
    # WRq[(i2,i3),(o2,o3),q=r24];  WR2[j,v,p,r] = sum_q WRq * c_root[p,q,r]
    wrq = np.einsum("ioc,jpd,cdq->ijopq", f2, f3, c_right, optimize=True).reshape(64, 64, 8)
    wr2 = np.einsum("jvq,pqr->jvpr", wrq, c_root, optimize=True)  # [j, v, p, r]

    # wlf [64, 512]: free = par*256 + c*64 + u  with  p = 2c + par
    wlf = np.ascontiguousarray(
        wl.reshape(64, 64, 4, 2).transpose(0, 3, 2, 1).reshape(64, 512)).astype(BF16_NP)
    # wr2all [128, 2048]: [par*64+j][c*512 + v*8 + r] = wr2[j, v, 2c+par, r]
    wr2c = wr2.transpose(2, 0, 1, 3).reshape(4, 2, 64, 64, 8).reshape(4, 128, 512)
    wr2all = np.ascontiguousarray(
        wr2c.transpose(1, 0, 2).reshape(128, 2048)).astype(BF16_NP)

    xf = x.reshape(B, 64, 64)
    xs = []
    for core in range(NCORES):
        xl = xf[core * BLOC:(core + 1) * BLOC]  # [64(b), 64(i), 64(j)]
        xs.append(np.ascontiguousarray(
            xl.transpose(1, 0, 2).reshape(64, BLOC * 64)).astype(BF16_NP))
    return xs, wlf, wr2all


def kernel(x, factors, cores, _want_profile=False):
    global _COMPILED
    x = np.asarray(x, dtype=np.float32)
    factors = np.asarray(factors, dtype=np.float32)
    cores = np.asarray(cores, dtype=np.float32)
    if _COMPILED is None:
        _COMPILED = _build()
    nc = _COMPILED
    xs, wlf, wr2all = _host_prep(x, factors, cores)
    in_maps = [{"x": xs[c], "wlf": wlf, "wr2all": wr2all} for c in range(NCORES)]
    res = run_bass_kernel_spmd(nc, in_maps, list(range(NCORES)), trace=_want_profile)
    out = np.concatenate(
        [np.asarray(res.results[c]["out"]).astype(np.float32).reshape(
            BLOC, 8, 8, 8, 8, 8) for c in range(NCORES)]
    )
    if _want_profile:
        return out, res
    return out
